# revision 20
# baseline (speedup 1.0000x reference)
"""Trainium2 Bass kernel for a GAT-style GNN layer (8 NeuronCores, SPMD).

Reference computation:
    h = x @ W                                  # [N, FOUT]
    e = leakyrelu(Wh1[row] + Wh2[col])         # per-edge scores
    att = softmax(e, axis=1)                   # axis of size 1 -> exactly 1.0
    out = elu(segment_sum(att * h[col], row))  # [N, FOUT]

Because the softmax is over a size-1 axis, att == 1.0 exactly, so the kernel
computes out = elu(segment_sum(h[col], row)) and `a` is unused.

Strategy (2 SPMD launches over 8 cores, sharded by destination-node range):
  Launch 1: h = x @ W for this core's node slice, stored as an interleaved
            dual-fp16 table h2[n] = [fp16(h[n]), fp16(h[n]-hi)] (~2^-22 rel).
  Host:     replicate h2 to all cores; bucket+sort edges by
            (dest window of 128 nodes, source chunk of 25000 nodes), pad each
            (window, chunk) group to a multiple of 128 and to the max over
            cores so all cores share one static program.
  Launch 2: dma_gather messages from h2; per 128-edge block build a one-hot
            S = (iota == row_local) and accumulate S.T @ msg into PSUM
            (segment sum); ELU; write the node-range slice of the output.
"""

import os
from contextlib import ExitStack
from dataclasses import dataclass, field

import ml_dtypes
import numpy as np

import concourse.bass as bass
import concourse.tile as tile
from concourse import bacc, library_config, mybir
from concourse.bass_utils import run_bass_kernel_spmd

F32 = mybir.dt.float32
F16 = mybir.dt.float16
BF16 = mybir.dt.bfloat16
I16 = mybir.dt.int16

P = 128


@dataclass(frozen=True)
class Config:
    n: int = 100000          # nodes
    fin: int = 256           # input features
    fout: int = 128          # output features
    ncores: int = 8
    nchunk: int = 4          # gather-table chunks (int16 index limit)
    gwin: int = 2            # windows per gather super-group
    dual: bool = True        # dual-fp16 (hi+lo) message table vs single fp16

    @property
    def npc(self):
        return self.n // self.ncores

    @property
    def nwin(self):
        return (self.npc + P - 1) // P

    @property
    def chunk(self):
        return self.n // self.nchunk

    @property
    def twidth(self):
        return (2 if self.dual else 1) * self.fout


CFG = Config()


def _split_hi_lo(x, dt):
    hi = x.astype(dt)
    lo = (x - hi.astype(np.float32)).astype(dt)
    return hi, lo


# --------------------------------------------------------------------------
# Static edge layout (shared across cores -> one SPMD program)
# --------------------------------------------------------------------------

@dataclass
class Layout:
    cfg: Config
    swc: np.ndarray          # [nwin*nchunk] padded group sizes (mult of 128)
    off: np.ndarray          # [nwin*nchunk] slot offset of each group
    runs: list               # [(g, c, start, length)] gather runs
    groups: list             # list of window-index lists
    total_slots: int = 0

    @property
    def nblocks(self):
        return self.total_slots // P


def build_layout(cfg: Config, row, col):
    """row/col: int arrays [E]. Returns (layout, per-core stream builders)."""
    npc, nwin, nchunk, chunk = cfg.npc, cfg.nwin, cfg.nchunk, cfg.chunk
    ngrp = nwin * nchunk

    per_core = []
    counts = np.zeros((cfg.ncores, ngrp), np.int64)
    for k in range(cfg.ncores):
        sel = (row >= k * npc) & (row < (k + 1) * npc)
        r = (row[sel] - k * npc).astype(np.int64)
        c_ = col[sel].astype(np.int64)
        w = r // P
        rl = r - w * P
        ch = c_ // chunk
        cl = c_ - ch * chunk
        key = w * nchunk + ch
        counts[k] = np.bincount(key, minlength=ngrp)
        per_core.append((key, cl, rl))

    swc = ((counts.max(axis=0) + P - 1) // P) * P  # padded static sizes

    groups = [list(range(g, min(g + cfg.gwin, nwin)))
              for g in range(0, nwin, cfg.gwin)]

    off = np.zeros(ngrp, np.int64)
    runs = []
    cur = 0
    for gi, g in enumerate(groups):
        for c in range(nchunk):
            start = cur
            for w in g:
                off[w * nchunk + c] = cur
                cur += swc[w * nchunk + c]
            runs.append((gi, c, start, cur - start))
    layout = Layout(cfg, swc, off, runs, groups, int(cur))
    return layout, per_core


def build_streams(layout: Layout, key, cl, rl):
    """Per-core edge streams: wrapped int16 gather indices + row-local values."""
    cfg = layout.cfg
    total = layout.total_slots
    idx_local = np.zeros(total, np.int16)
    rowloc = np.full(total, -1.0, np.float32)

    order = np.argsort(key, kind="stable")
    skey = key[order]
    scl = cl[order]
    srl = rl[order]
    cnt = np.bincount(key, minlength=len(layout.swc))
    starts = np.concatenate([[0], np.cumsum(cnt)[:-1]])
    rank = np.arange(len(skey)) - starts[skey]
    slot = layout.off[skey] + rank
    idx_local[slot] = scl.astype(np.int16)
    rowloc[slot] = srl

    idx_w = np.zeros((P, total // 16), np.int16)
    for (_, _, s, L) in layout.runs:
        if L == 0:
            continue
        seg = idx_local[s:s + L].reshape(-1, 16).T          # [16, L/16]
        idx_w[:, s // 16:(s + L) // 16] = np.tile(seg, (8, 1))
    rl_w = np.ascontiguousarray(
        rowloc.reshape(-1, P).T.astype(np.float16))          # [128, nblocks]
    return idx_w, rl_w


# --------------------------------------------------------------------------
# Launch 1: h = x @ W  (node-sharded), emit interleaved fp16 table
# --------------------------------------------------------------------------

def build_phase1(cfg: Config):
    nc = bacc.Bacc("TRN2", target_bir_lowering=False, debug=False,
                   num_devices=cfg.ncores)
    fin, fout, npc = cfg.fin, cfg.fout, cfg.npc
    nkt = fin // P
    xt_hi = nc.dram_tensor("xt_hi", [fin, npc], BF16, kind="ExternalInput")
    xt_lo = nc.dram_tensor("xt_lo", [fin, npc], BF16, kind="ExternalInput")
    w_hi = nc.dram_tensor("w_hi", [fin, fout], BF16, kind="ExternalInput")
    w_lo = nc.dram_tensor("w_lo", [fin, fout], BF16, kind="ExternalInput")
    h2 = nc.dram_tensor("h2", [npc, cfg.twidth], F16, kind="ExternalOutput")

    with tile.TileContext(nc) as tc, ExitStack() as ctx:
        wpool = ctx.enter_context(tc.tile_pool(name="w", bufs=1))
        xpool = ctx.enter_context(tc.tile_pool(name="x", bufs=1))
        ppool = ctx.enter_context(tc.tile_pool(name="ps", bufs=4, space="PSUM"))
        opool = ctx.enter_context(tc.tile_pool(name="o", bufs=4))

        whs, wls = [], []
        for k in range(nkt):
            wh = wpool.tile([P, fout], BF16, tag=f"wh{k}")
            wl = wpool.tile([P, fout], BF16, tag=f"wl{k}")
            nc.sync.dma_start(wh[:], w_hi.ap()[k * P:(k + 1) * P, :])
            nc.sync.dma_start(wl[:], w_lo.ap()[k * P:(k + 1) * P, :])
            whs.append(wh)
            wls.append(wl)

        # Whole x-transpose slice resident in SBUF: 4 big line-rate DMAs
        # instead of 4 small ones per node tile (HWDGE issue overhead).
        xh, xl = [], []
        for k in range(nkt):
            a = xpool.tile([P, npc], BF16, tag=f"xh{k}")
            b = xpool.tile([P, npc], BF16, tag=f"xl{k}")
            nc.sync.dma_start(a[:], xt_hi.ap()[k * P:(k + 1) * P, :])
            nc.sync.dma_start(b[:], xt_lo.ap()[k * P:(k + 1) * P, :])
            xh.append(a)
            xl.append(b)

        for t in range(cfg.nwin):
            n0 = t * P
            nt = min(npc - n0, P)
            ps = ppool.tile([P, fout], F32)
            mms = []
            for k in range(nkt):
                mms += [(xh[k], whs[k]), (xh[k], wls[k]), (xl[k], whs[k])]
            for j, (lhsT, rhs) in enumerate(mms):
                nc.tensor.matmul(ps[:nt, :], lhsT[:, n0:n0 + nt], rhs[:],
                                 start=(j == 0), stop=(j == len(mms) - 1))
            ot = opool.tile([P, cfg.twidth], F16, tag="ot")
            nc.vector.tensor_copy(ot[:nt, 0:fout], ps[:nt, :])
            if cfg.dual:
                t32 = opool.tile([P, fout], F32, tag="t32")
                nc.vector.tensor_copy(t32[:nt, :], ot[:nt, 0:fout])
                nc.vector.tensor_tensor(ot[:nt, fout:2 * fout], ps[:nt, :],
                                        t32[:nt, :], op=mybir.AluOpType.subtract)
            nc.sync.dma_start(h2.ap()[n0:n0 + nt, :], ot[:nt, :])
    nc.compile()
    return nc


# --------------------------------------------------------------------------
# Launch 2: gather + segment-sum (one-hot matmul) + ELU
# --------------------------------------------------------------------------

def build_phase2(cfg: Config, layout: Layout):
    nc = bacc.Bacc("TRN2", target_bir_lowering=False, debug=False,
                   num_devices=cfg.ncores, num_swdge_queues=4)
    fout, npc, nchunk, chunk = cfg.fout, cfg.npc, cfg.nchunk, cfg.chunk
    tw = cfg.twidth
    h2 = nc.dram_tensor("h2", [cfg.n, tw], F16, kind="ExternalInput")
    idxs = nc.dram_tensor("idxs", [P, layout.total_slots // 16], I16,
                          kind="ExternalInput")
    rowloc = nc.dram_tensor("rowloc", [P, layout.nblocks], F16,
                            kind="ExternalInput")
    SBATCH = 4  # one-hot builds batched over consecutive blocks
    iota_in = nc.dram_tensor("iota", [P, SBATCH * P], F16, kind="ExternalInput")
    out = nc.dram_tensor("out", [npc, fout], F32, kind="ExternalOutput")

    run_by_gc = {(g, c): (s, L) for (g, c, s, L) in layout.runs}
    bmax = {c: max((run_by_gc[(gi, c)][1] // P)
                   for gi in range(len(layout.groups))) for c in range(nchunk)}

    with tile.TileContext(nc) as tc, ExitStack() as ctx:
        nc.gpsimd.load_library(library_config.mlp)

        cpool = ctx.enter_context(tc.tile_pool(name="const", bufs=1))
        mpool = ctx.enter_context(tc.tile_pool(name="msg", bufs=6))
        spool = ctx.enter_context(tc.tile_pool(name="sel", bufs=6))
        ppool = ctx.enter_context(tc.tile_pool(name="ps", bufs=4, space="PSUM"))
        epool = ctx.enter_context(tc.tile_pool(name="elu", bufs=3))

        iota_t = cpool.tile([P, SBATCH * P], F16)
        nc.sync.dma_start(iota_t[:], iota_in.ap()[:, :])
        rl_t = cpool.tile([P, layout.nblocks], F16)
        nc.sync.dma_start(rl_t[:], rowloc.ap()[:, :])
        # whole idx stream resident: removes per-run DMAs + waits from the
        # gather issue path
        idx_t = cpool.tile([P, layout.total_slots // 16], I16)
        nc.sync.dma_start(idx_t[:], idxs.ap()[:, :])

        for gi, g in enumerate(layout.groups):
            mts = {}
            for c in range(nchunk):
                s, L = run_by_gc[(gi, c)]
                if L == 0:
                    continue
                mt = mpool.tile([P, bmax[c], tw], F16, tag=f"msg{c}")
                # single_packet=True (fast CounterMachine path) caps one
                # gather at 64 descs/engine = 1024 indices; split the run
                # into equal chunks (multiples of 128).
                nsub = (L + 1023) // 1024
                sub = -(-(L // P) // nsub) * P
                o = s
                while o < s + L:
                    Lg = min(sub, s + L - o)
                    nc.gpsimd.dma_gather(
                        mt[:, (o - s) // P:(o - s + Lg) // P, :],
                        h2.ap()[c * chunk:(c + 1) * chunk, :],
                        idx_t[:, o // 16:(o + Lg) // 16], Lg, Lg, tw,
                        single_packet=True, queue_num=c % 4)
                    o += Lg
                mts[c] = mt
            for w in g:
                blocks = []
                for c in range(nchunk):
                    o = layout.off[w * nchunk + c]
                    s, _ = run_by_gc[(gi, c)]
                    nb = layout.swc[w * nchunk + c] // P
                    for j in range(nb):
                        blocks.append((c, (o - s) // P + j, o // P + j))
                nt = min(npc - w * P, P)
                ot = epool.tile([P, fout], F32, tag="out")
                if not blocks:
                    nc.vector.memset(ot[:], 0.0)
                    nc.sync.dma_start(out.ap()[w * P:w * P + nt, :], ot[:nt, :])
                    continue
                ps = ppool.tile([P, fout], F32)
                nmm = len(blocks) * (2 if cfg.dual else 1)
                mi = 0
                # batch one-hot builds over runs of consecutive global blocks
                bi = 0
                while bi < len(blocks):
                    nb = 1
                    while (nb < SBATCH and bi + nb < len(blocks)
                           and blocks[bi + nb][2] == blocks[bi][2] + nb):
                        nb += 1
                    gb0 = blocks[bi][2]
                    sel = spool.tile([P, SBATCH * P], F16, tag="sel")
                    nc.vector.tensor_tensor(
                        sel[:, :nb * P], iota_t[:, :nb * P],
                        rl_t[:, gb0:gb0 + nb].to_broadcast([P, nb, P]),
                        op=mybir.AluOpType.is_equal)
                    for j in range(nb):
                        c, lb, _ = blocks[bi + j]
                        st = sel[:, j * P:(j + 1) * P]
                        nc.tensor.matmul(ps[:], st, mts[c][:, lb, 0:fout],
                                         start=(mi == 0), stop=(mi == nmm - 1))
                        mi += 1
                        if cfg.dual:
                            nc.tensor.matmul(ps[:], st,
                                             mts[c][:, lb, fout:2 * fout],
                                             start=False, stop=(mi == nmm - 1))
                            mi += 1
                    bi += nb
                # ELU: relu(x) - 1 + exp(min(x, 0))
                tmin = epool.tile([P, fout], F32, tag="tmin")
                texp = epool.tile([P, fout], F32, tag="texp")
                trel = epool.tile([P, fout], F32, tag="trel")
                nc.vector.tensor_scalar_min(tmin[:], ps[:], 0.0)
                nc.scalar.activation(texp[:], tmin[:],
                                     mybir.ActivationFunctionType.Exp)
                nc.vector.tensor_scalar(trel[:], ps[:], 0.0, -1.0,
                                        mybir.AluOpType.max,
                                        mybir.AluOpType.add)
                nc.vector.tensor_add(ot[:], texp[:], trel[:])
                nc.sync.dma_start(out.ap()[w * P:w * P + nt, :], ot[:nt, :])
    nc.compile()
    return nc


# --------------------------------------------------------------------------
# Host orchestration
# --------------------------------------------------------------------------

_P1_CACHE = {}
_P2_CACHE = {}


def _phase1_nc(cfg: Config):
    key = (cfg.n, cfg.fin, cfg.fout, cfg.ncores, cfg.dual)
    if key not in _P1_CACHE:
        _P1_CACHE[key] = build_phase1(cfg)
    return _P1_CACHE[key]


def _phase2_nc(cfg: Config, layout: Layout):
    key = (cfg.n, cfg.fin, cfg.fout, cfg.ncores, cfg.dual,
           tuple(layout.swc.tolist()))
    if key not in _P2_CACHE:
        _P2_CACHE[key] = build_phase2(cfg, layout)
    return _P2_CACHE[key]


def run(x, edge_index, W, a=None, cfg: Config = CFG, trace=False):
    """Full pipeline; returns (out, info dict with exec times)."""
    x = np.asarray(x, np.float32)
    W = np.asarray(W, np.float32)
    edge_index = np.asarray(edge_index)
    row = edge_index[0].astype(np.int64)
    col = edge_index[1].astype(np.int64)
    npc = cfg.npc
    info = {}

    # ---- phase 1 ----
    x_hi, x_lo = _split_hi_lo(x, ml_dtypes.bfloat16)
    w_hi, w_lo = _split_hi_lo(W, ml_dtypes.bfloat16)
    xt_hi = np.ascontiguousarray(x_hi.T)
    xt_lo = np.ascontiguousarray(x_lo.T)
    nc1 = _phase1_nc(cfg)
    in1 = [{
        "xt_hi": np.ascontiguousarray(xt_hi[:, k * npc:(k + 1) * npc]),
        "xt_lo": np.ascontiguousarray(xt_lo[:, k * npc:(k + 1) * npc]),
        "w_hi": w_hi, "w_lo": w_lo,
    } for k in range(cfg.ncores)]
    r1 = run_bass_kernel_spmd(nc1, in1, list(range(cfg.ncores)), trace=trace)
    h2 = np.concatenate([r1.results[k]["h2"] for k in range(cfg.ncores)], axis=0)
    info["p1_ns"] = r1.exec_time_ns

    # ---- layout + streams ----
    layout, per_core = build_layout(cfg, row, col)
    nc2 = _phase2_nc(cfg, layout)
    iota = np.ascontiguousarray(
        np.broadcast_to(np.tile(np.arange(P, dtype=np.float16), 4), (P, 4 * P)))
    in2 = []
    for k in range(cfg.ncores):
        idx_w, rl_w = build_streams(layout, *per_core[k])
        in2.append({"h2": h2, "idxs": idx_w, "rowloc": rl_w, "iota": iota})
    r2 = run_bass_kernel_spmd(nc2, in2, list(range(cfg.ncores)), trace=trace)
    out = np.concatenate([r2.results[k]["out"] for k in range(cfg.ncores)],
                         axis=0)
    info["p2_ns"] = r2.exec_time_ns
    info["total_slots"] = layout.total_slots
    info["results"] = (r1, r2)
    return out, info


def kernel(x, edge_index, W, a=None, **_ignored):
    out, _ = run(x, edge_index, W, a)
    return out
